# revision 5
# baseline (speedup 1.0000x reference)
"""LongRangeProj Bass kernel V2 for TRN2 (8 NeuronCores, channel-sharded).

Math per (b,c,o,pix):  s = sqa + rdn - lx   (minimized over o, then exp(-min))
  sqa = s2^2*(wc-1/2)^2 = s2^2*wc*(wc-1) + s2^2/4,  wc = frac(theta/2pi - a/2pi + 1/2)
  rdn = (sr*fn + br)^2,  sr = sqrt(inv2rv), br = -|rm|*sr,  lx = log(max(x,eps))
Identity used:  s2^2*wc(wc-1) + [s2^2/4 - lx]  folds amplitude into a per-
partition scalar, so the whole exponent is 2 transposable tiles (q12, rdn).
Origin-pixel mask (angle term := 0 there) is repaired post-reduce via a
host-built FIXT tile: min(o_t, br^2-lx at that origin's own pixel).

Engine assignment per channel (partitions = 2b x 64o, free = 4096 pixels):
  DVE   : wc = (V16 + c) mod 1  [ts 4x fp16]   (some channels on GPSIMD)
  GPSIMD: tmp = (wc - 1)*wc     [scalar_tensor_tensor]
  DVE   : q12 = s2sq*tmp + c2   [ts 4x fp16]   (some channels on ACT)
  ACT   : rdn = Square(sr*FN16 + br)
  PE    : psum16 += q12^T, rdn^T (128x128 fp16 transposes, accumulating)
  DVE   : min-reduce over origins from psum16, FIXT merge, ACT exp, DMA out
"""

import numpy as np
from contextlib import ExitStack

B, C, NH, NW, H, W = 2, 64, 8, 8, 64, 64
STRIDE = 8
NCORES = 8
CL = C // NCORES          # 8 channels per core
HW = H * W                # 4096
NO = NH * NW              # 64 origins
NBLK = HW // 128          # 32 pixel blocks
TWO_PI = 2.0 * np.pi
RMAGIC = 1536.0           # fp16 output-rounding magic offset
INF16 = 60000.0

# per-channel engine choice: wc on DVE ('V') or GPSIMD ('G');
# q12 on DVE ('V') or ACT ('A')
RR_ENG = ['G'] * 8          # rr16 + rr0 passes
SQ_ENG = ['A'] * 8
Q12_ENG = ['V', 'V', 'V', 'V', 'G', 'G', 'G', 'A']

W16 = 2 * HW + 128 + CL * NBLK * 2          # V16, FN16, I16, FIXT(8ch)
W32 = 7 * CL                    # ca, s2, s2a2, mlx, sra, bra, s2q

_built = {}


def _eng(tbl, it, h):
    e = tbl[it]
    return e if isinstance(e, str) and len(e) == 1 else e[h]


def _host_fields16():
    oy = np.arange(NH, dtype=np.float64) * STRIDE
    ox = np.arange(NW, dtype=np.float64) * STRIDE
    yg = np.arange(H, dtype=np.float64)
    xg = np.arange(W, dtype=np.float64)
    fy = yg[None, :] - oy[:, None]                      # [NH, H]
    fx = xg[None, :] - ox[:, None]                      # [NW, W]
    FY = np.broadcast_to(fy[:, None, :, None], (NH, NW, H, W))
    FX = np.broadcast_to(fx[None, :, None, :], (NH, NW, H, W))
    fn = np.sqrt(FX * FX + FY * FY)
    v = np.arctan2(FY, FX) / TWO_PI                     # [-0.5, 0.5]
    rs = lambda a: np.ascontiguousarray(a.reshape(NO, HW))
    v16 = rs(v).astype(np.float16)
    fn16 = rs(fn).astype(np.float16)
    return np.concatenate([v16, v16], 0), np.concatenate([fn16, fn16], 0)


def _build_bass():
    import concourse.bass as bass
    import concourse.bacc as bacc
    import concourse.tile as tile
    import concourse.mybir as mybir

    f32 = mybir.dt.float32
    f16 = mybir.dt.float16
    AF = mybir.ActivationFunctionType
    OP = mybir.AluOpType
    AX = mybir.AxisListType

    nc = bacc.Bacc("TRN2", target_bir_lowering=False)
    c16_d = nc.dram_tensor("c16", [128, W16], f16, kind="ExternalInput")
    c32_d = nc.dram_tensor("c32", [128, W32], f32, kind="ExternalInput")
    out_d = nc.dram_tensor("out", [2 * CL, HW], f32, kind="ExternalOutput")

    with ExitStack() as ctx:
        tc = ctx.enter_context(tile.TileContext(nc))
        cpool = ctx.enter_context(tc.tile_pool(name="const", bufs=1))
        work = ctx.enter_context(tc.tile_pool(name="work", bufs=4))
        psum = ctx.enter_context(tc.tile_pool(name="psum", bufs=2, space="PSUM"))
        outp = ctx.enter_context(tc.tile_pool(name="outp", bufs=2))

        V16t = cpool.tile([128, HW], f16, tag="V16")
        FN16t = cpool.tile([128, HW], f16, tag="FN16")
        I16t = cpool.tile([128, 128], f16, tag="I16")
        FIXTt = cpool.tile([128, CL * NBLK * 2], f16, tag="FIXT")
        C32 = cpool.tile([128, W32], f32, tag="C32")
        scratch = cpool.tile([128, 1], f16, tag="scratch")
        # separate tiles + separate DMA queues so consumers start ASAP
        nc.sync.dma_start(C32[:, :], c32_d[:, :])
        nc.sync.dma_start(V16t[:, 0:HW // 2], c16_d[:, 0:HW // 2])
        nc.sync.dma_start(V16t[:, HW // 2:], c16_d[:, HW // 2:HW])
        nc.scalar.dma_start(FN16t[:, 0:HW // 2], c16_d[:, HW:HW + HW // 2])
        nc.scalar.dma_start(FN16t[:, HW // 2:], c16_d[:, HW + HW // 2:2 * HW])
        nc.gpsimd.dma_start(I16t[:, :], c16_d[:, 2 * HW:2 * HW + 128])
        nc.gpsimd.dma_start(FIXTt[:, :], c16_d[:, 2 * HW + 128:])
        V16 = V16t[:, :]
        FN16 = FN16t[:, :]
        I32t = cpool.tile([128, 128], f32, tag="I32t")
        nc.vector.tensor_copy(I32t[:, :], I16t[:, :])
        I32 = I32t[:, :]
        FIXT = FIXTt[:, :].rearrange("p (c f) -> p c f", c=CL)
        CA = C32[:, 0 * CL:1 * CL]
        S2 = C32[:, 1 * CL:2 * CL]
        S2A2 = C32[:, 2 * CL:3 * CL]
        MLX = C32[:, 3 * CL:4 * CL]
        SRA = C32[:, 4 * CL:5 * CL]
        BRA = C32[:, 5 * CL:6 * CL]
        S2Q = C32[:, 6 * CL:7 * CL]

        NCH = 2
        HC = HW // NCH        # chunk pixels

        def emit_front(key):
            it, h = key
            sl = slice(h * HC, (h + 1) * HC)
            ca = CA[:, it:it + 1]
            s2 = S2[:, it:it + 1]
            s2a2 = S2A2[:, it:it + 1]
            mlx = MLX[:, it:it + 1]
            sra = SRA[:, it:it + 1]
            bra = BRA[:, it:it + 1]
            s2q = S2Q[:, it:it + 1]
            rdn = work.tile([128, HC], f32, tag="rdn")
            nc.scalar.activation(rdn[:, :], FN16[:, sl], AF.Square,
                                 scale=sra, bias=bra)
            # rr16 = 1536 + round(V + a2): fp16 output rounding
            reng = nc.vector if _eng(RR_ENG, it, h) == 'V' else nc.gpsimd
            rr = work.tile([128, HC], f16, tag="rr")
            reng.tensor_scalar(rr[:, :], V16[:, sl], ca, None, OP.add)
            # rr0 = round(V + a2), exactly representable in fp16
            rr0 = work.tile([128, HC], f16, tag="rr0")
            reng.tensor_scalar(rr0[:, :], rr[:, :], -1536.0, None, OP.add)
            # wq = V - rr0   (wu = wq + a2, folded into the square's bias)
            wq = work.tile([128, HC], f16, tag="wq")
            nc.vector.tensor_tensor(wq[:, :], V16[:, sl], rr0[:, :], OP.subtract)
            # sq = s2^2 * (wq + a2)^2
            sq = work.tile([128, HC], f32, tag="sq")
            if _eng(SQ_ENG, it, h) == 'A':
                nc.scalar.activation(sq[:, :], wq[:, :], AF.Square,
                                     scale=s2, bias=s2a2)
            else:
                aw = work.tile([128, HC], f16, tag="aw")
                nc.vector.tensor_scalar(aw[:, :], wq[:, :], s2, s2a2,
                                        OP.mult, OP.add)
                nc.vector.tensor_tensor(sq[:, :], aw[:, :], aw[:, :], OP.mult)
            # q12 = sq - lx
            q12 = work.tile([128, HC], f32, tag="q12")
            qe = _eng(Q12_ENG, it, h)
            if qe == 'V':
                nc.vector.tensor_scalar(q12[:, :], sq[:, :], mlx, None, OP.add)
            elif qe == 'G':
                nc.gpsimd.tensor_scalar(q12[:, :], sq[:, :], mlx, None, OP.add)
            else:
                nc.scalar.activation(q12[:, :], sq[:, :], AF.Identity, bias=mlx)
            return q12, rdn

        o_ts = {}

        def emit_tail(key, q12, rdn):
            it, h = key
            ps = psum.tile([128, HC], f32, tag="ps")
            for l in range(HC // 128):
                psl = slice(l * 128, (l + 1) * 128)
                nc.tensor.matmul(ps[:, psl], q12[:, psl], I32,
                                 start=True, stop=False, is_transpose=True)
                nc.tensor.matmul(ps[:, psl], rdn[:, psl], I32,
                                 start=False, stop=True, is_transpose=True)
            if h == 0:
                o_ts[it] = outp.tile([128, NBLK, 2], f16, tag="o_t", name=f"o_t_{it}")
            o_t = o_ts[it]
            nb = HC // 128
            nc.vector.tensor_reduce(
                o_t[:, h * nb:(h + 1) * nb, :],
                ps[:, :].rearrange("p (l r o) -> p l r o", l=nb, r=2),
                axis=AX.X, op=OP.min)
            if h == NCH - 1:
                o_m = outp.tile([128, NBLK, 2], f16, tag="o_m")
                nc.vector.tensor_tensor(o_m[:, :, :], o_t[:, :, :],
                                        FIXT[:, it, :]
                                        .rearrange("p (l r) -> p l r", r=2),
                                        OP.min)
                o_e = outp.tile([128, NBLK, 2], f32, tag="o_e")
                nc.scalar.activation(o_e[:, :, :], o_m[:, :, :], AF.Exp,
                                     scale=-1.0)
                for b in range(2):
                    nc.sync.dma_start(
                        out_d[b * CL + it].rearrange("(blk p) -> p blk", p=128),
                        o_e[:, :, b])

        keys = [(it, h) for it in range(CL) for h in range(NCH)]
        LEAD = globals().get('LEAD_N', 2 * NCH)
        fr = {}
        for key in keys[:LEAD]:
            fr[key] = emit_front(key)
        for i, key in enumerate(keys):
            if i + LEAD < len(keys):
                fr[keys[i + LEAD]] = emit_front(keys[i + LEAD])
            emit_tail(key, *fr.pop(key))
    nc.finalize()
    return nc


def _host_scalars(x, radius_mean, angle_mean, radius_std, angle_std):
    """Per-core fp32 tables [128, CL] (partition = b*64+o) and FIXT fp16."""
    inv2rv = 1.0 / (2.0 * (radius_std.astype(np.float64) ** 2 + 0.01))   # [C]
    inv2av = 1.0 / (2.0 * (angle_std.astype(np.float64) ** 2 + 0.0001))  # [C]
    s2sq = (TWO_PI ** 2) * inv2av                                        # [C]
    srt = np.sqrt(inv2rv)                                                # [C]
    rm = np.abs(radius_mean.astype(np.float64)).reshape(B, C, NO)
    am = angle_mean.astype(np.float64).reshape(B, C, NO)
    lx = np.log(np.maximum(x.astype(np.float64).reshape(B, C, NO), 1e-30))
    cores = []
    for k in range(NCORES):
        cs = np.arange(k * CL, (k + 1) * CL)
        ca = np.zeros((128, CL)); s2t = np.zeros((128, CL))
        s2a2 = np.zeros((128, CL)); mlxt = np.zeros((128, CL))
        sra = np.zeros((128, CL)); bra = np.zeros((128, CL))
        s2qt = np.zeros((128, CL))
        fixt = np.full((128, CL, NBLK, 2), INF16)
        for itc, c in enumerate(cs):
            s2c = np.sqrt(s2sq[c])
            for b in range(B):
                p = slice(b * NO, (b + 1) * NO)
                a2 = -am[b, c] / TWO_PI
                ca[p, itc] = a2 + RMAGIC
                s2t[p, itc] = s2c
                s2a2[p, itc] = s2c * a2
                mlxt[p, itc] = -lx[b, c]
                sra[p, itc] = srt[c]
                bra[p, itc] = -rm[b, c] * srt[c]
                s2qt[p, itc] = s2sq[c]
                for i in range(NH):
                    for j in range(NW):
                        o = i * NW + j
                        br2 = (rm[b, c, o] * srt[c]) ** 2
                        fixt[8 * j, itc, 4 * i, b] = min(br2 - lx[b, c, o], INF16)
                f = lambda a: np.ascontiguousarray(a.astype(np.float32))
        cores.append(dict(
            c32=np.ascontiguousarray(np.concatenate(
                [f(ca), f(s2t), f(s2a2), f(mlxt), f(sra), f(bra), f(s2qt)],
                axis=1)),
            fixt=np.ascontiguousarray(
                fixt.reshape(128, CL * NBLK * 2).astype(np.float16))))
    return cores


def _host_c16(fixt):
    v16, fn16 = _built["fields"]
    i16 = np.eye(128, dtype=np.float16)
    return np.ascontiguousarray(np.concatenate([v16, fn16, i16, fixt], axis=1))


def kernel(x, radius_mean, angle_mean, radius_std, angle_std):
    from concourse.bass_utils import run_bass_kernel_spmd

    if "nc" not in _built:
        _built["fields"] = _host_fields16()
        _built["nc"] = _build_bass()
    nc = _built["nc"]
    sc = _host_scalars(x, radius_mean, angle_mean, radius_std, angle_std)
    in_maps = [{"c16": _host_c16(s["fixt"]), "c32": s["c32"]} for s in sc]
    res = run_bass_kernel_spmd(nc, in_maps, core_ids=list(range(NCORES)))
    out = np.empty((B, C, H, W), dtype=np.float32)
    for k in range(NCORES):
        r = res.results[k]["out"].reshape(B, CL, H, W)
        out[:, k * CL:(k + 1) * CL] = r
    return out


# revision 6
# speedup vs baseline: 1.0736x; 1.0736x over previous
"""LongRangeProj Bass kernel V2 for TRN2 (8 NeuronCores, channel-sharded).

Math per (b,c,o,pix):  s = sqa + rdn - lx   (minimized over o, then exp(-min))
  sqa = s2^2*(wc-1/2)^2 = s2^2*wc*(wc-1) + s2^2/4,  wc = frac(theta/2pi - a/2pi + 1/2)
  rdn = (sr*fn + br)^2,  sr = sqrt(inv2rv), br = -|rm|*sr,  lx = log(max(x,eps))
Identity used:  s2^2*wc(wc-1) + [s2^2/4 - lx]  folds amplitude into a per-
partition scalar, so the whole exponent is 2 transposable tiles (q12, rdn).
Origin-pixel mask (angle term := 0 there) is repaired post-reduce via a
host-built FIXT tile: min(o_t, br^2-lx at that origin's own pixel).

Engine assignment per channel (partitions = 2b x 64o, free = 4096 pixels):
  DVE   : wc = (V16 + c) mod 1  [ts 4x fp16]   (some channels on GPSIMD)
  GPSIMD: tmp = (wc - 1)*wc     [scalar_tensor_tensor]
  DVE   : q12 = s2sq*tmp + c2   [ts 4x fp16]   (some channels on ACT)
  ACT   : rdn = Square(sr*FN16 + br)
  PE    : psum16 += q12^T, rdn^T (128x128 fp16 transposes, accumulating)
  DVE   : min-reduce over origins from psum16, FIXT merge, ACT exp, DMA out
"""

import numpy as np
from contextlib import ExitStack

B, C, NH, NW, H, W = 2, 64, 8, 8, 64, 64
STRIDE = 8
NCORES = 8
CL = C // NCORES          # 8 channels per core
HW = H * W                # 4096
NO = NH * NW              # 64 origins
NBLK = HW // 128          # 32 pixel blocks
TWO_PI = 2.0 * np.pi
RMAGIC = 1536.0           # fp16 output-rounding magic offset
INF16 = 60000.0

# per-channel engine choice: wc on DVE ('V') or GPSIMD ('G');
# q12 on DVE ('V') or ACT ('A')
RR_ENG = ['V', 'G', 'G', 'G', 'G', 'G', 'G', 'G']
SQ_ENG = ['A'] * 8
Q12_ENG = ['V', 'V', 'V', 'V', 'G', 'G', 'G', 'A']

W16 = 2 * HW + 128 + CL * NBLK * 2          # V16, FN16, I16, FIXT(8ch)
W32 = 7 * CL                    # ca, s2, s2a2, mlx, sra, bra, s2q

_built = {}


def _eng(tbl, it, h):
    e = tbl[it]
    return e if isinstance(e, str) and len(e) == 1 else e[h]


def _host_fields16():
    oy = np.arange(NH, dtype=np.float64) * STRIDE
    ox = np.arange(NW, dtype=np.float64) * STRIDE
    yg = np.arange(H, dtype=np.float64)
    xg = np.arange(W, dtype=np.float64)
    fy = yg[None, :] - oy[:, None]                      # [NH, H]
    fx = xg[None, :] - ox[:, None]                      # [NW, W]
    FY = np.broadcast_to(fy[:, None, :, None], (NH, NW, H, W))
    FX = np.broadcast_to(fx[None, :, None, :], (NH, NW, H, W))
    fn = np.sqrt(FX * FX + FY * FY)
    v = np.arctan2(FY, FX) / TWO_PI                     # [-0.5, 0.5]
    rs = lambda a: np.ascontiguousarray(a.reshape(NO, HW))
    v16 = rs(v).astype(np.float16)
    fn16 = rs(fn).astype(np.float16)
    return np.concatenate([v16, v16], 0), np.concatenate([fn16, fn16], 0)


def _build_bass():
    import concourse.bass as bass
    import concourse.bacc as bacc
    import concourse.tile as tile
    import concourse.mybir as mybir

    f32 = mybir.dt.float32
    f16 = mybir.dt.float16
    AF = mybir.ActivationFunctionType
    OP = mybir.AluOpType
    AX = mybir.AxisListType

    nc = bacc.Bacc("TRN2", target_bir_lowering=False)
    c16_d = nc.dram_tensor("c16", [128, W16], f16, kind="ExternalInput")
    c32_d = nc.dram_tensor("c32", [128, W32], f32, kind="ExternalInput")
    out_d = nc.dram_tensor("out", [2 * CL, HW], f32, kind="ExternalOutput")

    with ExitStack() as ctx:
        tc = ctx.enter_context(tile.TileContext(nc))
        cpool = ctx.enter_context(tc.tile_pool(name="const", bufs=1))
        work = ctx.enter_context(tc.tile_pool(name="work", bufs=4))
        psum = ctx.enter_context(tc.tile_pool(name="psum", bufs=2, space="PSUM"))
        outp = ctx.enter_context(tc.tile_pool(name="outp", bufs=2))

        V16t = cpool.tile([128, HW], f16, tag="V16")
        FN16t = cpool.tile([128, HW], f16, tag="FN16")
        I16t = cpool.tile([128, 128], f16, tag="I16")
        FIXTt = cpool.tile([128, CL * NBLK * 2], f16, tag="FIXT")
        C32 = cpool.tile([128, W32], f32, tag="C32")
        scratch = cpool.tile([128, 1], f16, tag="scratch")
        # separate tiles + separate DMA queues so consumers start ASAP
        nc.sync.dma_start(C32[:, :], c32_d[:, :])
        nc.sync.dma_start(V16t[:, 0:HW // 2], c16_d[:, 0:HW // 2])
        nc.sync.dma_start(V16t[:, HW // 2:], c16_d[:, HW // 2:HW])
        nc.scalar.dma_start(FN16t[:, 0:HW // 2], c16_d[:, HW:HW + HW // 2])
        nc.scalar.dma_start(FN16t[:, HW // 2:], c16_d[:, HW + HW // 2:2 * HW])
        nc.gpsimd.dma_start(I16t[:, :], c16_d[:, 2 * HW:2 * HW + 128])
        nc.gpsimd.dma_start(FIXTt[:, :], c16_d[:, 2 * HW + 128:])
        V16 = V16t[:, :]
        FN16 = FN16t[:, :]
        I32t = cpool.tile([128, 128], f32, tag="I32t")
        nc.vector.tensor_copy(I32t[:, :], I16t[:, :])
        I32 = I32t[:, :]
        FIXT = FIXTt[:, :].rearrange("p (c f) -> p c f", c=CL)
        CA = C32[:, 0 * CL:1 * CL]
        S2 = C32[:, 1 * CL:2 * CL]
        S2A2 = C32[:, 2 * CL:3 * CL]
        MLX = C32[:, 3 * CL:4 * CL]
        SRA = C32[:, 4 * CL:5 * CL]
        BRA = C32[:, 5 * CL:6 * CL]
        S2Q = C32[:, 6 * CL:7 * CL]

        NCH = 2
        HC = HW // NCH        # chunk pixels

        def emit_front(key):
            it, h = key
            sl = slice(h * HC, (h + 1) * HC)
            ca = CA[:, it:it + 1]
            s2 = S2[:, it:it + 1]
            s2a2 = S2A2[:, it:it + 1]
            mlx = MLX[:, it:it + 1]
            sra = SRA[:, it:it + 1]
            bra = BRA[:, it:it + 1]
            s2q = S2Q[:, it:it + 1]
            rdn = work.tile([128, HC], f32, tag="rdn")
            nc.scalar.activation(rdn[:, :], FN16[:, sl], AF.Square,
                                 scale=sra, bias=bra)
            # rr16 = 1536 + round(V + a2): fp16 output rounding
            reng = nc.vector if _eng(RR_ENG, it, h) == 'V' else nc.gpsimd
            rr = work.tile([128, HC], f16, tag="rr")
            reng.tensor_scalar(rr[:, :], V16[:, sl], ca, None, OP.add)
            # rr0 = round(V + a2), exactly representable in fp16
            rr0 = work.tile([128, HC], f16, tag="rr0")
            reng.tensor_scalar(rr0[:, :], rr[:, :], -1536.0, None, OP.add)
            # wq = V - rr0   (wu = wq + a2, folded into the square's bias)
            wq = work.tile([128, HC], f16, tag="wq")
            nc.vector.tensor_tensor(wq[:, :], V16[:, sl], rr0[:, :], OP.subtract)
            # sq = s2^2 * (wq + a2)^2
            sq = work.tile([128, HC], f32, tag="sq")
            if _eng(SQ_ENG, it, h) == 'A':
                nc.scalar.activation(sq[:, :], wq[:, :], AF.Square,
                                     scale=s2, bias=s2a2)
            else:
                aw = work.tile([128, HC], f16, tag="aw")
                nc.vector.tensor_scalar(aw[:, :], wq[:, :], s2, s2a2,
                                        OP.mult, OP.add)
                nc.vector.tensor_tensor(sq[:, :], aw[:, :], aw[:, :], OP.mult)
            # q12 = sq - lx
            q12 = work.tile([128, HC], f32, tag="q12")
            qe = _eng(Q12_ENG, it, h)
            if qe == 'V':
                nc.vector.tensor_scalar(q12[:, :], sq[:, :], mlx, None, OP.add)
            elif qe == 'G':
                nc.gpsimd.tensor_scalar(q12[:, :], sq[:, :], mlx, None, OP.add)
            else:
                nc.scalar.activation(q12[:, :], sq[:, :], AF.Identity, bias=mlx)
            return q12, rdn

        o_ts = {}

        def emit_tail(key, q12, rdn):
            it, h = key
            ps = psum.tile([128, HC], f32, tag="ps")
            for l in range(HC // 128):
                psl = slice(l * 128, (l + 1) * 128)
                nc.tensor.matmul(ps[:, psl], q12[:, psl], I32,
                                 start=True, stop=False, is_transpose=True)
                nc.tensor.matmul(ps[:, psl], rdn[:, psl], I32,
                                 start=False, stop=True, is_transpose=True)
            if h == 0:
                o_ts[it] = outp.tile([128, NBLK, 2], f16, tag="o_t", name=f"o_t_{it}")
            o_t = o_ts[it]
            nb = HC // 128
            nc.vector.tensor_reduce(
                o_t[:, h * nb:(h + 1) * nb, :],
                ps[:, :].rearrange("p (l r o) -> p l r o", l=nb, r=2),
                axis=AX.X, op=OP.min)
            if h == NCH - 1:
                o_m = outp.tile([128, NBLK, 2], f16, tag="o_m")
                nc.vector.tensor_tensor(o_m[:, :, :], o_t[:, :, :],
                                        FIXT[:, it, :]
                                        .rearrange("p (l r) -> p l r", r=2),
                                        OP.min)
                o_e = outp.tile([128, NBLK, 2], f32, tag="o_e")
                nc.scalar.activation(o_e[:, :, :], o_m[:, :, :], AF.Exp,
                                     scale=-1.0)
                for b in range(2):
                    nc.sync.dma_start(
                        out_d[b * CL + it].rearrange("(blk p) -> p blk", p=128),
                        o_e[:, :, b])

        keys = [(it, h) for it in range(CL) for h in range(NCH)]
        LEAD = globals().get('LEAD_N', 2 * NCH)
        fr = {}
        for key in keys[:LEAD]:
            fr[key] = emit_front(key)
        for i, key in enumerate(keys):
            if i + LEAD < len(keys):
                fr[keys[i + LEAD]] = emit_front(keys[i + LEAD])
            emit_tail(key, *fr.pop(key))
    nc.finalize()
    return nc


def _host_scalars(x, radius_mean, angle_mean, radius_std, angle_std):
    """Per-core fp32 tables [128, CL] (partition = b*64+o) and FIXT fp16."""
    inv2rv = 1.0 / (2.0 * (radius_std.astype(np.float64) ** 2 + 0.01))   # [C]
    inv2av = 1.0 / (2.0 * (angle_std.astype(np.float64) ** 2 + 0.0001))  # [C]
    s2sq = (TWO_PI ** 2) * inv2av                                        # [C]
    srt = np.sqrt(inv2rv)                                                # [C]
    rm = np.abs(radius_mean.astype(np.float64)).reshape(B, C, NO)
    am = angle_mean.astype(np.float64).reshape(B, C, NO)
    lx = np.log(np.maximum(x.astype(np.float64).reshape(B, C, NO), 1e-30))
    cores = []
    for k in range(NCORES):
        cs = np.arange(k * CL, (k + 1) * CL)
        ca = np.zeros((128, CL)); s2t = np.zeros((128, CL))
        s2a2 = np.zeros((128, CL)); mlxt = np.zeros((128, CL))
        sra = np.zeros((128, CL)); bra = np.zeros((128, CL))
        s2qt = np.zeros((128, CL))
        fixt = np.full((128, CL, NBLK, 2), INF16)
        for itc, c in enumerate(cs):
            s2c = np.sqrt(s2sq[c])
            for b in range(B):
                p = slice(b * NO, (b + 1) * NO)
                a2 = -am[b, c] / TWO_PI
                ca[p, itc] = a2 + RMAGIC
                s2t[p, itc] = s2c
                s2a2[p, itc] = s2c * a2
                mlxt[p, itc] = -lx[b, c]
                sra[p, itc] = srt[c]
                bra[p, itc] = -rm[b, c] * srt[c]
                s2qt[p, itc] = s2sq[c]
                for i in range(NH):
                    for j in range(NW):
                        o = i * NW + j
                        br2 = (rm[b, c, o] * srt[c]) ** 2
                        fixt[8 * j, itc, 4 * i, b] = min(br2 - lx[b, c, o], INF16)
                f = lambda a: np.ascontiguousarray(a.astype(np.float32))
        cores.append(dict(
            c32=np.ascontiguousarray(np.concatenate(
                [f(ca), f(s2t), f(s2a2), f(mlxt), f(sra), f(bra), f(s2qt)],
                axis=1)),
            fixt=np.ascontiguousarray(
                fixt.reshape(128, CL * NBLK * 2).astype(np.float16))))
    return cores


def _host_c16(fixt):
    v16, fn16 = _built["fields"]
    i16 = np.eye(128, dtype=np.float16)
    return np.ascontiguousarray(np.concatenate([v16, fn16, i16, fixt], axis=1))


def kernel(x, radius_mean, angle_mean, radius_std, angle_std):
    from concourse.bass_utils import run_bass_kernel_spmd

    if "nc" not in _built:
        _built["fields"] = _host_fields16()
        _built["nc"] = _build_bass()
    nc = _built["nc"]
    sc = _host_scalars(x, radius_mean, angle_mean, radius_std, angle_std)
    in_maps = [{"c16": _host_c16(s["fixt"]), "c32": s["c32"]} for s in sc]
    res = run_bass_kernel_spmd(nc, in_maps, core_ids=list(range(NCORES)))
    out = np.empty((B, C, H, W), dtype=np.float32)
    for k in range(NCORES):
        r = res.results[k]["out"].reshape(B, CL, H, W)
        out[:, k * CL:(k + 1) * CL] = r
    return out
